# revision 15
# baseline (speedup 1.0000x reference)
"""Trainium2 Bass kernel for nn_GaussianSplatter.

Strategy (data-parallel over batch, 2 images per core x 8 cores):
  launch 1: each core reduces its logits shard [2,100,112,112] to patch-position
            sums [100,49] (sum over batch and the 16x16 grid of 7x7 tiles).
  host:     combine the 8 partials -> lbar [49,100]; run the tiny Gaussian
            prototype math (covariance -> 5x5 kernels -> peak-norm -> pad ->
            bilinear translate -> opacity weight) exactly as the reference,
            producing W [49,49]; bake W into a block-diagonal stationary
            matrix LHS [112, 49, 112] so the 7x7 unfold/fold is pure matmul.
  launch 2: each core computes out[h',(img,nw,j)] = sum_h LHS[h,ccj,h']*feat[h,...]
            with 49 PE matmuls (PSUM-accumulated over cc), then writes the
            folded [2,8,112,112] shard.
"""
import numpy as np

import concourse.bass as bass
import concourse.mybir as mybir
from concourse.bass_utils import run_bass_kernel_spmd

N_CORES = 8
B, C_IN, H, W_ = 16, 64, 112, 112
K = 100
FEAT_C = 8
ROW = COL = 7
KSIZE = 5
P = 49
BPC = B // N_CORES          # batches per core
NIMG = BPC * FEAT_C         # images per core in launch 2
NCH = 28                    # h-rows per lambda1 chunk (4 nh groups)
NCHUNK = H // NCH           # 4

_cache = {}


def _translate_bilinear_np(img, sx, sy):
    Pn, Hh, Ww = img.shape
    ii = np.arange(Hh, dtype=img.dtype)[None, :, None] + sy[:, None, None]
    jj = np.arange(Ww, dtype=img.dtype)[None, None, :] + sx[:, None, None]
    ii = np.broadcast_to(ii, (Pn, Hh, Ww))
    jj = np.broadcast_to(jj, (Pn, Hh, Ww))
    i0 = np.floor(ii)
    j0 = np.floor(jj)
    wi = ii - i0
    wj = jj - j0
    pidx = np.arange(Pn)[:, None, None]

    def gather(iz, jz):
        valid = (iz >= 0) & (iz <= Hh - 1) & (jz >= 0) & (jz <= Ww - 1)
        v = img[pidx, np.clip(iz, 0, Hh - 1).astype(np.int32),
                np.clip(jz, 0, Ww - 1).astype(np.int32)]
        return np.where(valid, v, np.zeros((), img.dtype))

    v00 = gather(i0, j0)
    v01 = gather(i0, j0 + 1.0)
    v10 = gather(i0 + 1.0, j0)
    v11 = gather(i0 + 1.0, j0 + 1.0)
    return v00 * (1 - wi) * (1 - wj) + v01 * (1 - wi) * wj \
        + v10 * wi * (1 - wj) + v11 * wi * wj


def _compute_W(lbar, sigma_x, sigma_y, opacity, rho):
    lbar = lbar.astype(np.float64)
    wsx = lbar @ sigma_x.astype(np.float64)
    wsy = lbar @ sigma_y.astype(np.float64)
    wop = lbar @ opacity[:, 0].astype(np.float64)
    wrho = lbar @ rho[:, 0].astype(np.float64)
    a = wsx ** 2 + 1e-5
    d = wsy ** 2 + 1e-5
    b = wrho * wsx * wsy
    det = a * d - b * b
    ia, ib, idd = d / det, -b / det, a / det
    ax = np.linspace(-5.0, 5.0, KSIZE)
    xx = ax[:, None]
    yy = ax[None, :]
    z = -0.5 * (ia[:, None, None] * xx ** 2 + 2.0 * ib[:, None, None] * xx * yy
                + idd[:, None, None] * yy ** 2)
    kern = np.exp(z) / (2.0 * np.pi * np.sqrt(det)[:, None, None])
    kern = kern / kern.max(axis=(-2, -1), keepdims=True)
    ph, pw = ROW - KSIZE, COL - KSIZE
    kern = np.pad(kern, ((0, 0), (ph // 2, ph - ph // 2), (pw // 2, pw - pw // 2)))
    rr, cc = np.meshgrid(np.arange(ROW, dtype=np.float64),
                         np.arange(COL, dtype=np.float64), indexing='ij')
    tx = 1.0 - 2.0 * cc.reshape(-1) / ROW
    ty = 1.0 - 2.0 * rr.reshape(-1) / COL
    kT = _translate_bilinear_np(kern, tx * (COL - 1) / 2.0, ty * (ROW - 1) / 2.0)
    return (wop[:, None] * kT.reshape(P, P)).astype(np.float32)


def _build_blockdiag_lhs(W):
    """LHSJ[j, 7nh+r, cc, 7nh+i] = W[r*7+cc, i*7+j]; [7, 112, 7, 112] f32."""
    W4 = W.reshape(7, 7, 7, 7)                       # [r, cc, i, j]
    TJ = np.ascontiguousarray(W4.transpose(3, 0, 1, 2))  # [j, r, cc, i]
    LHS = np.zeros((7, 112, 7, 112), np.float32)
    for nh in range(16):
        LHS[:, 7 * nh:7 * nh + 7, :, 7 * nh:7 * nh + 7] = TJ
    return LHS


CHUNKS = [35, 35, 28, 14]             # h-rows per chunk (multiples of 7)


def _build_reduce_nc(chunks=None):
    chunks = chunks or CHUNKS
    nchunk = len(chunks)
    starts = np.concatenate([[0], np.cumsum(chunks)]).astype(int)
    assert starts[-1] == H
    nc = bass.Bass()
    f32 = mybir.dt.float32
    lg = nc.declare_dram_parameter("lg", [BPC, K, H, W_], f32, isOutput=False)
    partial = nc.declare_dram_parameter("partial", [nchunk, K, P], f32, isOutput=True)

    with bass.ExitStack() as ctx:
        A = ctx.enter_context(nc.sbuf_tensor([K, H, W_], f32))
        Bt = ctx.enter_context(nc.sbuf_tensor([K, H, W_], f32))
        Ct = ctx.enter_context(nc.sbuf_tensor([K, max(chunks), W_], f32))
        S1 = ctx.enter_context(nc.sbuf_tensor([K, 7, W_], f32))
        R2s = [ctx.enter_context(nc.sbuf_tensor(f"r2_{c}", [K, 7, 7], f32))
               for c in range(nchunk)]
        asems = [ctx.enter_context(nc.semaphore(f"a{c}")) for c in range(nchunk)]
        bsems = [ctx.enter_context(nc.semaphore(f"b{c}")) for c in range(nchunk)]
        vsems = [ctx.enter_context(nc.semaphore(f"v{c}")) for c in range(nchunk)]
        osem = ctx.enter_context(nc.semaphore("o"))
        block = ctx.enter_context(nc.Block())

        @block.sync
        def _(sync):
            for c in range(nchunk):
                sl = slice(starts[c], starts[c + 1])
                sync.dma_start(A[:, sl, :], lg[0, :, sl, :]).then_inc(asems[c], 16)
            for c in range(nchunk):
                sync.wait_ge(vsems[c], 1)
                sync.dma_start(partial[c], R2s[c][:]).then_inc(osem, 16)
            sync.wait_ge(osem, 16 * nchunk)

        @block.gpsimd
        def _(gpsimd):
            for c in range(nchunk):
                sl = slice(starts[c], starts[c + 1])
                gpsimd.dma_start(Bt[:, sl, :], lg[1, :, sl, :]).then_inc(bsems[c], 16)

        @block.vector
        def _(vector):
            for c in range(nchunk):
                sl = slice(starts[c], starts[c + 1])
                nhc = chunks[c] // 7
                vector.wait_ge(asems[c], 16)
                vector.wait_ge(bsems[c], 16)
                nch = chunks[c]
                nc.vector.tensor_add(Ct[:, :nch, :], A[:, sl, :], Bt[:, sl, :])
                nc.vector.reduce_sum(
                    S1[:], Ct[:, :nch, :].rearrange("k (nh r) w -> k r w nh", nh=nhc),
                    axis=mybir.AxisListType.X)
                nc.vector.reduce_sum(
                    R2s[c][:], S1[:].rearrange("k r (nw cc) -> k r cc nw", cc=7),
                    axis=mybir.AxisListType.X).then_inc(vsems[c], 1)
    return nc


def _build_apply_nc():
    nc = bass.Bass()
    f32 = mybir.dt.float32
    f32r = mybir.dt.float32r
    feat = nc.declare_dram_parameter("feat", [BPC, FEAT_C, H, W_], f32, isOutput=False)
    lhs = nc.declare_dram_parameter("lhs", [7, 112, 7, 112], f32, isOutput=False)
    y = nc.declare_dram_parameter("y", [BPC, FEAT_C, H, W_], f32, isOutput=True)

    with bass.ExitStack() as ctx:
        S4 = ctx.enter_context(nc.sbuf_tensor([112, NIMG, 16, 7], f32r))
        LHS = ctx.enter_context(nc.sbuf_tensor([112, 7, 7, 112], f32r))
        OUT = ctx.enter_context(nc.sbuf_tensor([112, NIMG, 16, 7], f32))
        psums = [ctx.enter_context(nc.psum_tensor(f"ps{j}", [112, 512], f32))
                 for j in range(7)]
        dsem = ctx.enter_context(nc.semaphore("d"))
        lsems = [ctx.enter_context(nc.semaphore(f"l{j}")) for j in range(7)]
        msem = ctx.enter_context(nc.semaphore("m"))
        csem = ctx.enter_context(nc.semaphore("c"))
        osem = ctx.enter_context(nc.semaphore("o"))
        block = ctx.enter_context(nc.Block())

        @block.sync
        def _(sync):
            sync.dma_start(
                S4[:], feat[:].rearrange("b c h (nw cc) -> h (b c) nw cc", cc=7).bitcast(f32r)
            ).then_inc(dsem, 16)
            sync.wait_ge(csem, 7)
            sync.dma_start(
                y[:].rearrange("b c h (nw cc) -> h (b c) nw cc", cc=7), OUT[:]
            ).then_inc(osem, 16)
            sync.wait_ge(osem, 16)

        @block.gpsimd
        def _(gpsimd):
            for j in range(7):
                gpsimd.dma_start(LHS[:, j, :, :], lhs[j].bitcast(f32r)).then_inc(lsems[j], 16)

        @block.tensor
        def _(tensor):
            tensor.wait_ge(dsem, 16)
            for j in range(7):
                tensor.wait_ge(lsems[j], 16)
                for cc in range(7):
                    ins = nc.tensor.matmul(
                        psums[j][:, 0:NIMG * 16],
                        LHS[:, j, cc, :],
                        S4[:, :, :, cc],
                        start=(cc == 0), stop=(cc == 6))
                    if cc == 6:
                        ins.then_inc(msem, 1)

        @block.scalar
        def _(scalar):
            for j in range(7):
                scalar.wait_ge(msem, j + 1)
                nc.scalar.copy(
                    OUT[:, :, :, j],
                    psums[j][:, 0:NIMG * 16].rearrange("p (i n) -> p i n", i=NIMG)
                ).then_inc(csem, 1)
    return nc


def kernel(inp, logits, sigma_x, sigma_y, opacity, rho, scale):
    inp = np.asarray(inp)
    logits = np.ascontiguousarray(np.asarray(logits, dtype=np.float32))
    feat = np.ascontiguousarray(np.asarray(inp[:, :FEAT_C], dtype=np.float32))

    if "reduce" not in _cache:
        _cache["reduce"] = _build_reduce_nc()
    if "apply" not in _cache:
        _cache["apply"] = _build_apply_nc()

    core_ids = list(range(N_CORES))
    in_maps1 = [{"lg": logits[BPC * i:BPC * (i + 1)]} for i in core_ids]
    res1 = run_bass_kernel_spmd(_cache["reduce"], in_maps1, core_ids)

    lbar_sum = np.zeros((K, P), np.float64)
    for i in core_ids:
        lbar_sum += res1.results[i]["partial"].astype(np.float64).sum(axis=0)
    lbar = (lbar_sum / (B * 16 * 16)).T          # [49, 100]

    Wm = _compute_W(lbar, np.asarray(sigma_x), np.asarray(sigma_y),
                    np.asarray(opacity), np.asarray(rho))
    LHS = _build_blockdiag_lhs(Wm)

    in_maps2 = [{"feat": feat[BPC * i:BPC * (i + 1)], "lhs": LHS} for i in core_ids]
    res2 = run_bass_kernel_spmd(_cache["apply"], in_maps2, core_ids)

    out = np.concatenate([res2.results[i]["y"] for i in core_ids], axis=0)
    return out.astype(np.float32)


# revision 18
# speedup vs baseline: 1.0163x; 1.0163x over previous
"""Trainium2 Bass kernel for nn_GaussianSplatter.

Strategy (data-parallel over batch, 2 images per core x 8 cores):
  launch 1: each core reduces its logits shard [2,100,112,112] to patch-position
            sums [100,49] (sum over batch and the 16x16 grid of 7x7 tiles).
  host:     combine the 8 partials -> lbar [49,100]; run the tiny Gaussian
            prototype math (covariance -> 5x5 kernels -> peak-norm -> pad ->
            bilinear translate -> opacity weight) exactly as the reference,
            producing W [49,49]; bake W into a block-diagonal stationary
            matrix LHS [112, 49, 112] so the 7x7 unfold/fold is pure matmul.
  launch 2: each core computes out[h',(img,nw,j)] = sum_h LHS[h,ccj,h']*feat[h,...]
            with 49 PE matmuls (PSUM-accumulated over cc), then writes the
            folded [2,8,112,112] shard.
"""
import numpy as np

import concourse.bass as bass
import concourse.mybir as mybir
from concourse.bass_utils import run_bass_kernel_spmd

N_CORES = 8
B, C_IN, H, W_ = 16, 64, 112, 112
K = 100
FEAT_C = 8
ROW = COL = 7
KSIZE = 5
P = 49
BPC = B // N_CORES          # batches per core
NIMG = BPC * FEAT_C         # images per core in launch 2
NCH = 28                    # h-rows per lambda1 chunk (4 nh groups)
NCHUNK = H // NCH           # 4

_cache = {}


def _translate_bilinear_np(img, sx, sy):
    Pn, Hh, Ww = img.shape
    ii = np.arange(Hh, dtype=img.dtype)[None, :, None] + sy[:, None, None]
    jj = np.arange(Ww, dtype=img.dtype)[None, None, :] + sx[:, None, None]
    ii = np.broadcast_to(ii, (Pn, Hh, Ww))
    jj = np.broadcast_to(jj, (Pn, Hh, Ww))
    i0 = np.floor(ii)
    j0 = np.floor(jj)
    wi = ii - i0
    wj = jj - j0
    pidx = np.arange(Pn)[:, None, None]

    def gather(iz, jz):
        valid = (iz >= 0) & (iz <= Hh - 1) & (jz >= 0) & (jz <= Ww - 1)
        v = img[pidx, np.clip(iz, 0, Hh - 1).astype(np.int32),
                np.clip(jz, 0, Ww - 1).astype(np.int32)]
        return np.where(valid, v, np.zeros((), img.dtype))

    v00 = gather(i0, j0)
    v01 = gather(i0, j0 + 1.0)
    v10 = gather(i0 + 1.0, j0)
    v11 = gather(i0 + 1.0, j0 + 1.0)
    return v00 * (1 - wi) * (1 - wj) + v01 * (1 - wi) * wj \
        + v10 * wi * (1 - wj) + v11 * wi * wj


def _compute_W(lbar, sigma_x, sigma_y, opacity, rho):
    lbar = lbar.astype(np.float64)
    wsx = lbar @ sigma_x.astype(np.float64)
    wsy = lbar @ sigma_y.astype(np.float64)
    wop = lbar @ opacity[:, 0].astype(np.float64)
    wrho = lbar @ rho[:, 0].astype(np.float64)
    a = wsx ** 2 + 1e-5
    d = wsy ** 2 + 1e-5
    b = wrho * wsx * wsy
    det = a * d - b * b
    ia, ib, idd = d / det, -b / det, a / det
    ax = np.linspace(-5.0, 5.0, KSIZE)
    xx = ax[:, None]
    yy = ax[None, :]
    z = -0.5 * (ia[:, None, None] * xx ** 2 + 2.0 * ib[:, None, None] * xx * yy
                + idd[:, None, None] * yy ** 2)
    kern = np.exp(z) / (2.0 * np.pi * np.sqrt(det)[:, None, None])
    kern = kern / kern.max(axis=(-2, -1), keepdims=True)
    ph, pw = ROW - KSIZE, COL - KSIZE
    kern = np.pad(kern, ((0, 0), (ph // 2, ph - ph // 2), (pw // 2, pw - pw // 2)))
    rr, cc = np.meshgrid(np.arange(ROW, dtype=np.float64),
                         np.arange(COL, dtype=np.float64), indexing='ij')
    tx = 1.0 - 2.0 * cc.reshape(-1) / ROW
    ty = 1.0 - 2.0 * rr.reshape(-1) / COL
    kT = _translate_bilinear_np(kern, tx * (COL - 1) / 2.0, ty * (ROW - 1) / 2.0)
    return (wop[:, None] * kT.reshape(P, P)).astype(np.float32)


def _build_blockdiag_lhs(W):
    """LHSJ[j, 7nh+r, cc, 7nh+i] = W[r*7+cc, i*7+j]; [7, 112, 7, 112] f32."""
    W4 = W.reshape(7, 7, 7, 7)                       # [r, cc, i, j]
    TJ = np.ascontiguousarray(W4.transpose(3, 0, 1, 2))  # [j, r, cc, i]
    LHS = np.zeros((7, 112, 7, 112), np.float32)
    for nh in range(16):
        LHS[:, 7 * nh:7 * nh + 7, :, 7 * nh:7 * nh + 7] = TJ
    return LHS


CHUNKS = [35, 35, 28, 14]             # h-rows per chunk (multiples of 7)


def _build_reduce_nc(chunks=None):
    chunks = chunks or CHUNKS
    nchunk = len(chunks)
    starts = np.concatenate([[0], np.cumsum(chunks)]).astype(int)
    assert starts[-1] == H
    nc = bass.Bass()
    f32 = mybir.dt.float32
    lg = nc.declare_dram_parameter("lg", [BPC, K, H, W_], f32, isOutput=False)
    partial = nc.declare_dram_parameter("partial", [nchunk, K, P], f32, isOutput=True)

    with bass.ExitStack() as ctx:
        A = ctx.enter_context(nc.sbuf_tensor([K, H, W_], f32))
        Bt = ctx.enter_context(nc.sbuf_tensor([K, H, W_], f32))
        Ct = ctx.enter_context(nc.sbuf_tensor([K, max(chunks), W_], f32))
        S1 = ctx.enter_context(nc.sbuf_tensor([K, 7, W_], f32))
        R2s = [ctx.enter_context(nc.sbuf_tensor(f"r2_{c}", [K, 7, 7], f32))
               for c in range(nchunk)]
        asems = [ctx.enter_context(nc.semaphore(f"a{c}")) for c in range(nchunk)]
        bsems = [ctx.enter_context(nc.semaphore(f"b{c}")) for c in range(nchunk)]
        vsems = [ctx.enter_context(nc.semaphore(f"v{c}")) for c in range(nchunk)]
        gsems = [ctx.enter_context(nc.semaphore(f"g{c}")) for c in range(nchunk)]
        ctsems = [ctx.enter_context(nc.semaphore(f"ct{c}")) for c in range(nchunk)]
        osem = ctx.enter_context(nc.semaphore("o"))
        block = ctx.enter_context(nc.Block())

        @block.sync
        def _(sync):
            for c in range(nchunk):
                sl = slice(starts[c], starts[c + 1])
                sync.dma_start(A[:, sl, :], lg[0, :, sl, :]).then_inc(asems[c], 16)
            for c in range(nchunk):
                sync.wait_ge(vsems[c], 1)
                sync.dma_start(partial[c], R2s[c][:]).then_inc(osem, 16)
            sync.wait_ge(osem, 16 * nchunk)

        @block.gpsimd
        def _(gpsimd):
            for c in range(nchunk):
                sl = slice(starts[c], starts[c + 1])
                gpsimd.dma_start(Bt[:, sl, :], lg[1, :, sl, :]).then_inc(bsems[c], 16)
            for c in range(nchunk):
                h2 = chunks[c] // 2
                sg = slice(starts[c] + h2, starts[c + 1])
                gpsimd.wait_ge(asems[c], 16)
                gpsimd.wait_ge(bsems[c], 16)
                if c > 0:
                    gpsimd.wait_ge(ctsems[c - 1], 1)
                nc.gpsimd.tensor_add(Ct[:, h2:chunks[c], :], A[:, sg, :],
                                     Bt[:, sg, :]).then_inc(gsems[c], 1)

        @block.vector
        def _(vector):
            for c in range(nchunk):
                sl = slice(starts[c], starts[c + 1])
                nhc = chunks[c] // 7
                vector.wait_ge(asems[c], 16)
                vector.wait_ge(bsems[c], 16)
                nch = chunks[c]
                h2 = nch // 2
                sv = slice(starts[c], starts[c] + h2)
                nc.vector.tensor_add(Ct[:, :h2, :], A[:, sv, :], Bt[:, sv, :])
                vector.wait_ge(gsems[c], 1)
                nc.vector.reduce_sum(
                    S1[:], Ct[:, :nch, :].rearrange("k (nh r) w -> k r w nh", nh=nhc),
                    axis=mybir.AxisListType.X).then_inc(ctsems[c], 1)
                nc.vector.reduce_sum(
                    R2s[c][:], S1[:].rearrange("k r (nw cc) -> k r cc nw", cc=7),
                    axis=mybir.AxisListType.X).then_inc(vsems[c], 1)
    return nc


def _build_apply_nc():
    nc = bass.Bass()
    f32 = mybir.dt.float32
    f32r = mybir.dt.float32r
    feat = nc.declare_dram_parameter("feat", [BPC, FEAT_C, H, W_], f32, isOutput=False)
    lhs = nc.declare_dram_parameter("lhs", [7, 112, 7, 112], f32, isOutput=False)
    y = nc.declare_dram_parameter("y", [BPC, FEAT_C, H, W_], f32, isOutput=True)

    with bass.ExitStack() as ctx:
        S4 = ctx.enter_context(nc.sbuf_tensor([112, NIMG, 16, 7], f32r))
        LHS = ctx.enter_context(nc.sbuf_tensor([112, 7, 7, 112], f32r))
        OUT = ctx.enter_context(nc.sbuf_tensor([112, NIMG, 16, 7], f32))
        psums = [ctx.enter_context(nc.psum_tensor(f"ps{j}", [112, 512], f32))
                 for j in range(7)]
        dsem = ctx.enter_context(nc.semaphore("d"))
        lsems = [ctx.enter_context(nc.semaphore(f"l{j}")) for j in range(7)]
        msem = ctx.enter_context(nc.semaphore("m"))
        csem = ctx.enter_context(nc.semaphore("c"))
        osem = ctx.enter_context(nc.semaphore("o"))
        block = ctx.enter_context(nc.Block())

        @block.sync
        def _(sync):
            sync.dma_start(
                S4[:], feat[:].rearrange("b c h (nw cc) -> h (b c) nw cc", cc=7).bitcast(f32r)
            ).then_inc(dsem, 16)
            sync.wait_ge(csem, 7)
            sync.dma_start(
                y[:].rearrange("b c h (nw cc) -> h (b c) nw cc", cc=7), OUT[:]
            ).then_inc(osem, 16)
            sync.wait_ge(osem, 16)

        @block.gpsimd
        def _(gpsimd):
            for j in range(7):
                gpsimd.dma_start(LHS[:, j, :, :], lhs[j].bitcast(f32r)).then_inc(lsems[j], 16)

        @block.tensor
        def _(tensor):
            tensor.wait_ge(dsem, 16)
            for j in range(7):
                tensor.wait_ge(lsems[j], 16)
                for cc in range(7):
                    ins = nc.tensor.matmul(
                        psums[j][:, 0:NIMG * 16],
                        LHS[:, j, cc, :],
                        S4[:, :, :, cc],
                        start=(cc == 0), stop=(cc == 6))
                    if cc == 6:
                        ins.then_inc(msem, 1)

        @block.scalar
        def _(scalar):
            for j in range(7):
                scalar.wait_ge(msem, j + 1)
                nc.scalar.copy(
                    OUT[:, :, :, j],
                    psums[j][:, 0:NIMG * 16].rearrange("p (i n) -> p i n", i=NIMG)
                ).then_inc(csem, 1)
    return nc


def kernel(inp, logits, sigma_x, sigma_y, opacity, rho, scale):
    inp = np.asarray(inp)
    logits = np.ascontiguousarray(np.asarray(logits, dtype=np.float32))
    feat = np.ascontiguousarray(np.asarray(inp[:, :FEAT_C], dtype=np.float32))

    if "reduce" not in _cache:
        _cache["reduce"] = _build_reduce_nc()
    if "apply" not in _cache:
        _cache["apply"] = _build_apply_nc()

    core_ids = list(range(N_CORES))
    in_maps1 = [{"lg": logits[BPC * i:BPC * (i + 1)]} for i in core_ids]
    res1 = run_bass_kernel_spmd(_cache["reduce"], in_maps1, core_ids)

    lbar_sum = np.zeros((K, P), np.float64)
    for i in core_ids:
        lbar_sum += res1.results[i]["partial"].astype(np.float64).sum(axis=0)
    lbar = (lbar_sum / (B * 16 * 16)).T          # [49, 100]

    Wm = _compute_W(lbar, np.asarray(sigma_x), np.asarray(sigma_y),
                    np.asarray(opacity), np.asarray(rho))
    LHS = _build_blockdiag_lhs(Wm)

    in_maps2 = [{"feat": feat[BPC * i:BPC * (i + 1)], "lhs": LHS} for i in core_ids]
    res2 = run_bass_kernel_spmd(_cache["apply"], in_maps2, core_ids)

    out = np.concatenate([res2.results[i]["y"] for i in core_ids], axis=0)
    return out.astype(np.float32)
